# revision 36
# baseline (speedup 1.0000x reference)
"""Bass/Trainium2 kernel for nn_DevConv_48060684042822 (gnn_message_passing).

out[i] = prev[i] + mean_d( W_phi[d] * max_{k<16} ((nodes[dst[i,k]] - nodes[i]) @ W_theta)[d] )

Math: with y = nodes @ W_theta [N, 32], per node i:
    maxi[i, :] = (max_k y[dst[i,k], :]) - y[i, :]
    out[i] = prev[i] + (1/32) * sum_d W_phi[d] * maxi[i, d]
edge_src = repeat(arange(N), 16) (sorted, fixed degree) so edges shard
perfectly across 8 cores along node blocks; no collective needed.

Device algorithm per core (12500 nodes, 200000 edges):
  Phase A: y table [ntab, 32] fp16 built by PE matmuls (full table replicated
           per core), stored in core-local HBM as 4-node packs [ntab/4, 128]
           (256B rows).  Also y_own [128, cols*32] f32 for this core's nodes.
  Phase B: 49x dma_gather (4096 idx each, idx = dst>>2 int16, 4 SWDGE
           queues, 8 buffers), 1-of-4 pack select via additive -30000 bias
           (host-uploaded) + max folds, then a 4-level pairwise max tree
           over the 16 neighbour slots.
  Phase C: (maxg - y_own) * (W_phi/32), sum over features, add prev, DMA out.
Host does index preprocessing + sharding only.
"""

import numpy as np

import concourse.bass as bass
import concourse.tile as tile
from concourse import bacc, mybir

N_NODES = 100000
DEGREE = 16
OUT_DIM = 32
N_CORES = 8

F16 = mybir.dt.float16
F32 = mybir.dt.float32
F32R = mybir.dt.float32r
I16 = mybir.dt.int16


class Cfg:
    """Geometry. All shapes derived from (n_nodes, nodes_per_core, tile_nodes)."""

    def __init__(self, n_nodes=N_NODES, nodes_per_core=12500, tile_nodes=256):
        self.n_nodes = n_nodes
        self.npc = nodes_per_core
        self.tn = tile_nodes                      # nodes per gather tile
        assert tile_nodes % 128 == 0
        self.mblk = tile_nodes // 128             # m-blocks per tile
        self.nt = -(-nodes_per_core // tile_nodes)  # gather tiles per core
        self.npc_pad = self.nt * tile_nodes       # padded nodes per core
        self.nidx = tile_nodes * DEGREE           # gather indices per tile
        assert self.nidx % 128 == 0 and self.nidx <= 8192
        self.cblk = self.nidx // 128              # G columns per partition
        self.ntab = -(-n_nodes // 512) * 512      # padded y-table rows
        self.qrows = self.ntab // 4               # 4-node packs
        assert self.qrows - 1 < 2 ** 15
        self.cols = self.nt * self.mblk           # output cols per partition
        self.ntiles_a = self.ntab // 128          # phase-A node tiles
        self.chunk_tiles = 96                     # node-tiles per nodesT chunk
        self.nchunks = -(-self.ntiles_a // self.chunk_tiles)
        assert self.ntab % 1024 == 0              # 8-tile write windows


def build_bass(cfg: Cfg, n_cores=N_CORES, repeat_a=1, repeat_b=1, repeat_full=1, skip_dve=False, skip_gather=False, gbufs=8, cbufs=2, scratch=16384):
    nc = bacc.Bacc("TRN2", target_bir_lowering=False, debug=False,
                   num_devices=n_cores, num_swdge_queues=4,
                   dynamic_dma_scratch_size=scratch)
    D = OUT_DIM
    # node features pre-arranged for K-stacked matmuls: 8 groups of 3 features
    # stacked into K=24 so one matmul emits y for 1024 (table) / 1024 (own)
    # nodes against a block-diagonal [24, 256] weights matrix.
    nodesK = nc.dram_tensor("nodesK", [24, cfg.ntab // 8], F32R,
                            kind="ExternalInput").ap()
    ngo = -(-cfg.cols // 8)                       # 8-tile y_own groups
    ownK = nc.dram_tensor("ownK", [24, ngo * 128], F32R,
                          kind="ExternalInput").ap()
    wtheta = nc.dram_tensor("wtheta", [24, 8 * D], F32R, kind="ExternalInput").ap()
    wphi = nc.dram_tensor("wphi", [128, D], F16, kind="ExternalInput").ap()
    # per-tile [idx | mask] packed in one tensor: idx int16 (nidx//16 words),
    # then the f16 lane-bias mask duplicated over feature-pairs (so every
    # tensor_tensor operand has a contiguous innermost dim -> DVE 2x mode)
    iw = cfg.nidx // 16
    mw = cfg.cblk * 4 * 2
    imsk_d = nc.dram_tensor("imsk", [cfg.nt, 128, iw + mw], I16,
                            kind="ExternalInput").ap()
    prev_d = nc.dram_tensor("prevp", [128, cfg.cols], F32, kind="ExternalInput").ap()
    out_d = nc.dram_tensor("outp", [128, cfg.cols], F32, kind="ExternalOutput").ap()

    with tile.TileContext(nc) as tc:
        for _rf in range(repeat_full):
          with tc.tile_pool(name="dram", bufs=1, space="DRAM") as dpool:
            ytab = dpool.tile([cfg.qrows, 128], F16, name=f"ytab{_rf}")
            yrows = ytab[:].rearrange("q (s d) -> (q s) d", d=D)

            with tc.tile_pool(name="persist", bufs=1) as pers:
                yown = pers.tile([128, cfg.cols * D], F16)
                maxg = pers.tile([128, cfg.cols * D], F16)
                if skip_dve:
                    nc.gpsimd.memset(maxg[:], 0.0)

                # ---------------- Phase A: y table + y_own ----------------
                for _ra in range(repeat_a):
                  with (
                      tc.tile_pool(name="aconst", bufs=1) as cpool,
                      tc.tile_pool(name="achunk", bufs=2) as npool,
                      tc.tile_pool(name="apsum", bufs=4, space="PSUM") as ppool,
                      tc.tile_pool(name="aystage", bufs=4) as spool,
                  ):
                      wt = cpool.tile([24, 8 * D], F32R)
                      nc.sync.dma_start(wt[:], wtheta[:])
                      # K-stacked windows: one [24,128]x[24,256] matmul emits y
                      # for 1024 nodes; out col j*32+d, partition p -> node
                      # 8p+j, so partition p holds 8 consecutive nodes ->
                      # 512B table-write descriptors. 4 windows per DMA.
                      wind_per_chunk = cfg.chunk_tiles // 8
                      nwind_tot = cfg.ntiles_a // 8
                      for ch in range(cfg.nchunks):
                          w0 = ch * wind_per_chunk
                          w1 = min(w0 + wind_per_chunk, nwind_tot)
                          ntile = npool.tile([24, (w1 - w0) * 128], F32R,
                                             name=f"nch{ch}", tag="nch")
                          nc.sync.dma_start(ntile[:], nodesK[:, w0 * 128:w1 * 128])
                          for wg in range(-(-(w1 - w0) // 4)):
                              wlo = wg * 4
                              whi = min(wlo + 4, w1 - w0)
                              ys = spool.tile([128, (whi - wlo) * 8 * D], F16,
                                              name=f"ys{ch}_{wg}", tag="ys")
                              # window pairs share one PSUM bank -> one
                              # PSUM->SBUF copy per 2048 nodes, alternating
                              # between the ACT and (idle) DVE engines
                              for wp in range(-(-(whi - wlo) // 2)):
                                  plo = wlo + wp * 2
                                  phi = min(plo + 2, whi)
                                  ps = ppool.tile([128, (phi - plo) * 256], F32,
                                                  name=f"ps{ch}_{plo}",
                                                  tag="ps", space="PSUM")
                                  for w in range(plo, phi):
                                      nc.tensor.matmul(
                                          ps[:, (w - plo) * 256:(w - plo + 1) * 256],
                                          ntile[:, w * 128:(w + 1) * 128],
                                          wt[:], start=True, stop=True)
                                  ydst = ys[:, (plo - wlo) * 256:(phi - wlo) * 256]
                                  if wp % 2 == 0:
                                      nc.scalar.copy(ydst, ps[:])
                                  else:
                                      nc.vector.tensor_copy(ydst, ps[:])
                              lo = (w0 + wlo) * 1024
                              hi = (w0 + whi) * 1024
                              dst = yrows[lo:hi, :].rearrange(
                                  "(w p j) d -> p w j d", p=128, j=8)
                              nc.sync.dma_start(dst, ys[:].rearrange(
                                  "p (w j d) -> p w j d", j=8, d=D))
                      # y_own: this core's nodes, [128, cols*D] f16; group g
                      # stacks tiles 8g..8g+7 (K rows c'*3+f), out col c'*32+d
                      otile = npool.tile([24, ngo * 128], F32R, name="ochunk",
                                         tag="nch")
                      nc.sync.dma_start(otile[:], ownK[:])
                      for g in range(ngo):
                          nt_g = min(8, cfg.cols - g * 8)
                          ps = ppool.tile([128, 512], F32, name=f"pso{g}",
                                          tag="pso", space="PSUM")
                          nc.tensor.matmul(
                              ps[:, :nt_g * D],
                              otile[:3 * nt_g, g * 128:(g + 1) * 128],
                              wt[:3 * nt_g, :nt_g * D],
                              start=True, stop=True)
                          nc.scalar.copy(yown[:, g * 8 * D:(g * 8 + nt_g) * D],
                                         ps[:, :nt_g * D])

                # ---------- Phase B: gather + select + max tree ----------
                for _rb in range(repeat_b):
                  with (
                      tc.tile_pool(name="bidx", bufs=4) as ipool,
                      tc.tile_pool(name="bg", bufs=gbufs) as gpool,
                      tc.tile_pool(name="bv", bufs=cbufs) as vpool,
                      tc.tile_pool(name="btree", bufs=cbufs) as tpool,
                  ):
                      for t in range(cfg.nt):
                          imt = ipool.tile([128, iw + mw], I16,
                                           name=f"imt{t}", tag="imt")
                          nc.sync.dma_start(imt[:], imsk_d[t])
                          it = imt[:][:, :iw]
                          mt = imt[:].bitcast(F16)[:, iw:iw + mw]
                          g_t = gpool.tile([128, cfg.cblk * 4 * D], F16,
                                           name=f"g{t}", tag="g")
                          if skip_gather:
                              nc.sync.dma_start(
                                  g_t[:].bitcast(I16)[:, :cfg.nidx // 16],
                                  imsk_d[t][:, :iw])
                          else:
                              nc.gpsimd.dma_gather(
                                  out_ap=g_t[:].rearrange("p (c e) -> p c e", e=4 * D),
                                  in_ap=ytab[:],
                                  idxs_ap=it,
                                  num_idxs=cfg.nidx,
                                  num_idxs_reg=cfg.nidx,
                                  elem_size=4 * D,
                                  single_packet=False,
                                  queue_num=t % 4,
                              )
                          if not skip_dve:
                              # [p, (c g), d2, dp] views: innermost contiguous
                              # pair on every operand -> DVE 2x mode
                              gvp = g_t[:].rearrange(
                                  "p (cg d2 dp) -> p cg d2 dp", d2=D // 2, dp=2)
                              biasp = mt.rearrange(
                                  "p (cg o dp) -> p cg o dp", o=1, dp=2) \
                                  .broadcast_to([128, cfg.cblk * 4, D // 2, 2])
                              gb = vpool.tile([128, cfg.cblk * 4 * D], F16,
                                              name=f"gb{t}", tag="gb")
                              gbp = gb[:].rearrange(
                                  "p (cg d2 dp) -> p cg d2 dp", d2=D // 2, dp=2)
                              nc.vector.tensor_tensor(gbp, gvp, biasp,
                                                      op=mybir.AluOpType.add)
                              gbv = gb[:].rearrange("p (c g d) -> p c g d", g=4, d=D)
                              s2 = vpool.tile([128, cfg.cblk * 2 * D], F16,
                                              name=f"s2{t}", tag="s2")
                              s2v = s2[:].rearrange("p (c g d) -> p c g d", g=2, d=D)
                              nc.vector.tensor_tensor(s2v, gbv[:, :, 0:2, :],
                                                      gbv[:, :, 2:4, :],
                                                      op=mybir.AluOpType.max)
                              v = vpool.tile([128, cfg.cblk * D], F16, name=f"v{t}", tag="v")
                              vv = v[:].rearrange("p (c d) -> p c d", d=D)
                              nc.vector.tensor_tensor(vv, s2v[:, :, 0, :], s2v[:, :, 1, :],
                                                      op=mybir.AluOpType.max)
                              v4 = v[:].rearrange("p (m k d) -> p m k d", k=DEGREE, d=D)
                              t1_ = tpool.tile([128, cfg.mblk * 8 * D], F16,
                                               name=f"t1_{t}", tag="t1")
                              t1v = t1_[:].rearrange("p (m k d) -> p m k d", k=8, d=D)
                              nc.vector.tensor_tensor(t1v, v4[:, :, 0:8, :],
                                                      v4[:, :, 8:16, :],
                                                      op=mybir.AluOpType.max)
                              t2_ = tpool.tile([128, cfg.mblk * 4 * D], F16,
                                               name=f"t2_{t}", tag="t2")
                              t2v = t2_[:].rearrange("p (m k d) -> p m k d", k=4, d=D)
                              nc.vector.tensor_tensor(t2v, t1v[:, :, 0:4, :],
                                                      t1v[:, :, 4:8, :],
                                                      op=mybir.AluOpType.max)
                              t3_ = tpool.tile([128, cfg.mblk * 2 * D], F16,
                                               name=f"t3_{t}", tag="t3")
                              t3v = t3_[:].rearrange("p (m k d) -> p m k d", k=2, d=D)
                              nc.vector.tensor_tensor(t3v, t2v[:, :, 0:2, :],
                                                      t2v[:, :, 2:4, :],
                                                      op=mybir.AluOpType.max)
                              mslice = maxg[:].rearrange("p (c d) -> p c d", d=D) \
                                  [:, t * cfg.mblk:(t + 1) * cfg.mblk, :]
                              nc.vector.tensor_tensor(mslice, t3v[:, :, 0, :],
                                                      t3v[:, :, 1, :],
                                                      op=mybir.AluOpType.max)

                # ---------------- Phase C: combine ----------------
                with tc.tile_pool(name="cpool", bufs=1) as cp:
                    wp = cp.tile([128, D], F16)
                    nc.sync.dma_start(wp[:], wphi[:])
                    pv = cp.tile([128, cfg.cols], F32)
                    nc.sync.dma_start(pv[:], prev_d[:])
                    diff = cp.tile([128, cfg.cols * D], F16)
                    nc.vector.tensor_tensor(diff[:], maxg[:], yown[:],
                                            op=mybir.AluOpType.subtract)
                    prod = cp.tile([128, cfg.cols * D], F16)
                    pvv = prod[:].rearrange("p (c d) -> p c d", d=D)
                    wpb = wp[:].rearrange("p (o d) -> p o d", o=1) \
                        .broadcast_to([128, cfg.cols, D])
                    nc.vector.tensor_tensor(
                        pvv, diff[:].rearrange("p (c d) -> p c d", d=D),
                        wpb, op=mybir.AluOpType.mult)
                    red = cp.tile([128, cfg.cols], F32)
                    nc.vector.tensor_reduce(
                        red[:].rearrange("p (c o) -> p c o", o=1), pvv,
                        axis=mybir.AxisListType.X, op=mybir.AluOpType.add)
                    outt = cp.tile([128, cfg.cols], F32)
                    nc.vector.tensor_tensor(outt[:], red[:], pv[:],
                                            op=mybir.AluOpType.add)
                    nc.sync.dma_start(out_d[:], outt[:])

    nc.compile()
    return nc


def host_prep(cfg: Cfg, edge_dst, nodes, W_theta, W_phi, prev, n_cores=N_CORES):
    """Build per-core input maps. Index preprocessing + padding only."""
    N = cfg.n_nodes
    dst = np.asarray(edge_dst).astype(np.int64).reshape(N, DEGREE)
    nodes = np.asarray(nodes, dtype=np.float32)
    npad = np.zeros((cfg.ntab, 3), dtype=np.float32)
    npad[:N] = nodes
    # K-stacked layout: row j*3+f, col w*128+p  <-  node 1024w+8p+j, feat f
    nodesK = np.ascontiguousarray(
        npad.reshape(cfg.ntab // 1024, 128, 8, 3)
        .transpose(2, 3, 0, 1).reshape(24, cfg.ntab // 8))
    wth = np.asarray(W_theta, dtype=np.float32)
    wtheta = np.zeros((24, 8 * OUT_DIM), dtype=np.float32)
    for j in range(8):
        wtheta[3 * j:3 * j + 3, OUT_DIM * j:OUT_DIM * (j + 1)] = wth
    wphi = np.tile(np.asarray(W_phi, dtype=np.float32)[None, :] / OUT_DIM,
                   (128, 1)).astype(np.float16)
    ngo = -(-cfg.cols // 8)

    in_maps = []
    for c in range(n_cores):
        lo = c * cfg.npc
        dst_c = np.zeros((cfg.npc_pad, DEGREE), dtype=np.int64)
        dst_c[:cfg.npc] = dst[lo:lo + cfg.npc]
        # A[t, m, k, p] = dst_c[tn*t + 128m + p, k]
        A = dst_c.reshape(cfg.nt, cfg.mblk, 128, DEGREE).transpose(0, 1, 3, 2)
        flat = A.reshape(cfg.nt, cfg.nidx)        # j = (m*DEG + k)*128 + p
        quad = (flat >> 2).astype(np.int16)
        sub = (flat & 3).astype(np.int16)
        iw = quad.reshape(cfg.nt, cfg.nidx // 16, 16).transpose(0, 2, 1)
        idx_w = np.tile(iw, (1, 8, 1)).astype(np.int16)
        sub_pc = sub.reshape(cfg.nt, cfg.cblk, 128).transpose(0, 2, 1)
        # bias[t, p, c, g] = 0 where g == dst&3 else -30000; duplicated over
        # feature-pairs so the kernel-side AP has a contiguous innermost dim
        bias = np.where(sub_pc[:, :, :, None] == np.arange(4)[None, None, None, :],
                        np.float16(0), np.float16(-30000))
        msk = np.repeat(bias.reshape(cfg.nt, 128, cfg.cblk * 4), 2,
                        axis=2).astype(np.float16)
        imsk = np.concatenate([idx_w, msk.view(np.int16)], axis=2)
        own = np.zeros((ngo * 1024, 3), dtype=np.float32)
        hi = min(lo + cfg.npc_pad, N)
        own[:hi - lo] = nodes[lo:hi]
        # group g stacks tiles 8g..8g+7: row c'*3+f, col g*128+p
        ownK = np.ascontiguousarray(
            own.reshape(ngo, 8, 128, 3).transpose(1, 3, 0, 2)
            .reshape(24, ngo * 128))
        prevp = np.zeros((128, cfg.cols), dtype=np.float32)
        pcv = np.zeros(cfg.npc_pad, dtype=np.float32)
        pcv[:cfg.npc] = prev[lo:lo + cfg.npc]
        prevp[:, :] = pcv.reshape(cfg.cols, 128).T
        in_maps.append({
            "nodesK": nodesK, "ownK": ownK,
            "wtheta": wtheta, "wphi": wphi,
            "imsk": np.ascontiguousarray(imsk),
            "prevp": prevp,
        })
    return in_maps


def assemble(cfg: Cfg, results, n_cores=N_CORES):
    out = np.empty(n_cores * cfg.npc, dtype=np.float32)
    for c in range(n_cores):
        arr = results[c]["outp"]            # [128, cols]
        out[c * cfg.npc:(c + 1) * cfg.npc] = arr.T.ravel()[:cfg.npc]
    return out


_CACHE = {}


def _make_exec(cfg: Cfg, n_cores=N_CORES, **build_kw):
    """Compile once and build a reusable jitted executable for the SPMD run."""
    import jax
    from jax.sharding import Mesh, PartitionSpec
    from jax.experimental.shard_map import shard_map
    from concourse import bass2jax

    nc = build_bass(cfg, n_cores, **build_kw)
    bass2jax.install_neuronx_cc_hook()
    in_names, out_names, out_avals, zero_shapes = [], [], [], []
    partition_name = nc.partition_id_tensor.name if nc.partition_id_tensor else None
    for alloc in nc.m.functions[0].allocations:
        if not isinstance(alloc, mybir.MemoryLocationSet):
            continue
        name = alloc.memorylocations[0].name
        if alloc.kind == "ExternalInput":
            if name != partition_name:
                in_names.append(name)
        elif alloc.kind == "ExternalOutput":
            out_names.append(name)
            shape = tuple(alloc.tensor_shape)
            dtype = mybir.dt.np(alloc.dtype)
            out_avals.append(jax.core.ShapedArray(shape, dtype))
            zero_shapes.append((shape, dtype))
    n_params = len(in_names)
    n_outs = len(out_avals)
    in_names_all = in_names + out_names
    if partition_name is not None:
        in_names_all.append(partition_name)
    donate = tuple(range(n_params, n_params + n_outs))

    def _body(*args):
        operands = list(args)
        if partition_name is not None:
            operands.append(bass2jax.partition_id_tensor())
        outs = bass2jax._bass_exec_p.bind(
            *operands,
            out_avals=tuple(out_avals),
            in_names=tuple(in_names_all),
            out_names=tuple(out_names),
            lowering_input_output_aliases=(),
            sim_require_finite=True,
            sim_require_nnan=True,
            nc=nc,
        )
        return tuple(outs)

    devices = jax.devices()[:n_cores]
    mesh = Mesh(np.asarray(devices), ("core",))
    in_specs = (PartitionSpec("core"),) * (n_params + n_outs)
    out_specs = (PartitionSpec("core"),) * len(out_names)
    sharded = jax.jit(
        shard_map(_body, mesh=mesh, in_specs=in_specs, out_specs=out_specs,
                  check_rep=False),
        donate_argnums=donate, keep_unused=True,
    )

    def run(in_maps):
        concat_in = [np.concatenate([np.asarray(m[name]) for m in in_maps], axis=0)
                     for name in in_names]
        zeros = [np.zeros((n_cores * s[0], *s[1:]), d) for s, d in zero_shapes]
        outs = sharded(*concat_in, *zeros)
        return [
            {name: np.asarray(outs[i]).reshape(n_cores, *out_avals[i].shape)[c]
             for i, name in enumerate(out_names)}
            for c in range(n_cores)
        ]

    return run


def _get_exec(cfg: Cfg, n_cores=N_CORES, **build_kw):
    key = (cfg.n_nodes, cfg.npc, cfg.tn, n_cores, tuple(sorted(build_kw.items())))
    if key not in _CACHE:
        _CACHE[key] = _make_exec(cfg, n_cores, **build_kw)
    return _CACHE[key]


def kernel(previous_inclusion_score, nodes, W_phi, W_theta, edge_src, edge_dst):
    cfg = Cfg()
    src = np.asarray(edge_src).astype(np.int64)
    expect = np.repeat(np.arange(N_NODES, dtype=np.int64), DEGREE)
    assert np.array_equal(src, expect), "edge_src structure mismatch"
    run = _get_exec(cfg)
    in_maps = host_prep(cfg, edge_dst, nodes, W_theta, W_phi,
                        np.asarray(previous_inclusion_score, dtype=np.float32))
    results = run(in_maps)
    return assemble(cfg, results)



# revision 41
# speedup vs baseline: 1.8623x; 1.8623x over previous
"""Bass/Trainium2 kernel for nn_DevConv_48060684042822 (gnn_message_passing).

out[i] = prev[i] + mean_d( W_phi[d] * max_{k<16} ((nodes[dst[i,k]] - nodes[i]) @ W_theta)[d] )

Math: with y = nodes @ W_theta [N, 32], per node i:
    maxi[i, :] = (max_k y[dst[i,k], :]) - y[i, :]
    out[i] = prev[i] + (1/32) * sum_d W_phi[d] * maxi[i, d]
edge_src = repeat(arange(N), 16) (sorted, fixed degree) so edges shard
perfectly across 8 cores along node blocks; no collective needed.

Device algorithm per core (12500 nodes, 200000 edges):
  Phase A: y table [ntab, 32] fp16 built by PE matmuls (full table replicated
           per core), stored in core-local HBM as 4-node packs [ntab/4, 128]
           (256B rows).  Also y_own [128, cols*32] f32 for this core's nodes.
  Phase B: 49x dma_gather (4096 idx each, idx = dst>>2 int16, 4 SWDGE
           queues, 8 buffers), 1-of-4 pack select via additive -30000 bias
           (host-uploaded) + max folds, then a 4-level pairwise max tree
           over the 16 neighbour slots.
  Phase C: (maxg - y_own) * (W_phi/32), sum over features, add prev, DMA out.
Host does index preprocessing + sharding only.
"""

import numpy as np

import concourse.bass as bass
import concourse.tile as tile
from concourse import bacc, mybir

N_NODES = 100000
DEGREE = 16
OUT_DIM = 32
N_CORES = 8

F16 = mybir.dt.float16
F32 = mybir.dt.float32
F32R = mybir.dt.float32r
I16 = mybir.dt.int16


class Cfg:
    """Geometry. All shapes derived from (n_nodes, nodes_per_core, tile_nodes)."""

    def __init__(self, n_nodes=N_NODES, nodes_per_core=12500, tile_nodes=256):
        self.n_nodes = n_nodes
        self.npc = nodes_per_core
        self.tn = tile_nodes                      # nodes per gather tile
        assert tile_nodes % 128 == 0
        self.mblk = tile_nodes // 128             # m-blocks per tile
        self.nt = -(-nodes_per_core // tile_nodes)  # gather tiles per core
        self.npc_pad = self.nt * tile_nodes       # padded nodes per core
        self.nidx = tile_nodes * DEGREE           # gather indices per tile
        assert self.nidx % 128 == 0 and self.nidx <= 8192
        self.cblk = self.nidx // 128              # G columns per partition
        self.ntab = -(-n_nodes // 512) * 512      # padded y-table rows
        self.qrows = self.ntab // 4               # 4-node packs
        assert self.qrows - 1 < 2 ** 15
        self.cols = self.nt * self.mblk           # output cols per partition
        self.ntiles_a = self.ntab // 128          # phase-A node tiles
        self.chunk_tiles = 96                     # node-tiles per nodesT chunk
        self.nchunks = -(-self.ntiles_a // self.chunk_tiles)
        assert self.ntab % 1024 == 0              # 8-tile write windows


def build_bass(cfg: Cfg, n_cores=N_CORES, repeat_a=1, repeat_b=1, repeat_full=1, skip_dve=False, skip_gather=False, gbufs=8, cbufs=2, scratch=16384):
    nc = bacc.Bacc("TRN2", target_bir_lowering=False, debug=False,
                   num_devices=n_cores, num_swdge_queues=4,
                   dynamic_dma_scratch_size=scratch)
    D = OUT_DIM
    # node features pre-arranged for K-stacked matmuls: 8 groups of 3 features
    # stacked into K=24 so one matmul emits y for 1024 (table) / 1024 (own)
    # nodes against a block-diagonal [24, 256] weights matrix.
    nodesK = nc.dram_tensor("nodesK", [24, cfg.ntab // 8], F32R,
                            kind="ExternalInput").ap()
    ngo = -(-cfg.cols // 8)                       # 8-tile y_own groups
    ownK = nc.dram_tensor("ownK", [24, ngo * 128], F32R,
                          kind="ExternalInput").ap()
    wtheta = nc.dram_tensor("wtheta", [24, 8 * D], F32R, kind="ExternalInput").ap()
    wphi = nc.dram_tensor("wphi", [128, D], F16, kind="ExternalInput").ap()
    # per-tile [idx | mask] packed in one tensor: idx int16 (nidx//16 words),
    # then the f16 lane-bias mask duplicated over feature-pairs (so every
    # tensor_tensor operand has a contiguous innermost dim -> DVE 2x mode)
    iw = cfg.nidx // 16
    mw = cfg.cblk * 4 * 2
    imsk_d = nc.dram_tensor("imsk", [128, cfg.nt * (iw + mw)], I16,
                            kind="ExternalInput").ap()
    prev_d = nc.dram_tensor("prevp", [128, cfg.cols], F32, kind="ExternalInput").ap()
    out_d = nc.dram_tensor("outp", [128, cfg.cols], F32, kind="ExternalOutput").ap()

    with tile.TileContext(nc) as tc:
        for _rf in range(repeat_full):
          with tc.tile_pool(name="dram", bufs=1, space="DRAM") as dpool:
            ytab = dpool.tile([cfg.qrows, 128], F16, name=f"ytab{_rf}")
            yrows = ytab[:].rearrange("q (s d) -> (q s) d", d=D)

            with tc.tile_pool(name="persist", bufs=1) as pers:
                yown = pers.tile([128, cfg.cols * D], F16)
                maxg = pers.tile([128, cfg.cols * D], F16)
                # all per-tile [idx|mask] data prefetched in one DMA that
                # overlaps the phase-A table build
                imt_all = pers.tile([128, cfg.nt * (iw + mw)], I16)
                nc.sync.dma_start(imt_all[:], imsk_d[:])
                if skip_dve:
                    nc.gpsimd.memset(maxg[:], 0.0)

                # ---------------- Phase A: y table + y_own ----------------
                for _ra in range(repeat_a):
                  with (
                      tc.tile_pool(name="aconst", bufs=1) as cpool,
                      tc.tile_pool(name="achunk", bufs=2) as npool,
                      tc.tile_pool(name="apsum", bufs=4, space="PSUM") as ppool,
                      tc.tile_pool(name="aystage", bufs=4) as spool,
                  ):
                      wt = cpool.tile([24, 8 * D], F32R)
                      nc.sync.dma_start(wt[:], wtheta[:])
                      # K-stacked windows: one [24,128]x[24,256] matmul emits y
                      # for 1024 nodes; out col j*32+d, partition p -> node
                      # 8p+j, so partition p holds 8 consecutive nodes ->
                      # 512B table-write descriptors. 4 windows per DMA.
                      wind_per_chunk = cfg.chunk_tiles // 8
                      nwind_tot = cfg.ntiles_a // 8
                      for ch in range(cfg.nchunks):
                          w0 = ch * wind_per_chunk
                          w1 = min(w0 + wind_per_chunk, nwind_tot)
                          ntile = npool.tile([24, (w1 - w0) * 128], F32R,
                                             name=f"nch{ch}", tag="nch")
                          nc.sync.dma_start(ntile[:], nodesK[:, w0 * 128:w1 * 128])
                          for wg in range(-(-(w1 - w0) // 4)):
                              wlo = wg * 4
                              whi = min(wlo + 4, w1 - w0)
                              ys = spool.tile([128, (whi - wlo) * 8 * D], F16,
                                              name=f"ys{ch}_{wg}", tag="ys")
                              # window pairs share one PSUM bank -> one
                              # PSUM->SBUF copy per 2048 nodes, alternating
                              # between the ACT and (idle) DVE engines
                              for wp in range(-(-(whi - wlo) // 2)):
                                  plo = wlo + wp * 2
                                  phi = min(plo + 2, whi)
                                  ps = ppool.tile([128, (phi - plo) * 256], F32,
                                                  name=f"ps{ch}_{plo}",
                                                  tag="ps", space="PSUM")
                                  for w in range(plo, phi):
                                      nc.tensor.matmul(
                                          ps[:, (w - plo) * 256:(w - plo + 1) * 256],
                                          ntile[:, w * 128:(w + 1) * 128],
                                          wt[:], start=True, stop=True)
                                  ydst = ys[:, (plo - wlo) * 256:(phi - wlo) * 256]
                                  if wp % 2 == 0:
                                      nc.scalar.copy(ydst, ps[:])
                                  else:
                                      nc.vector.tensor_copy(ydst, ps[:])
                              lo = (w0 + wlo) * 1024
                              hi = (w0 + whi) * 1024
                              dst = yrows[lo:hi, :].rearrange(
                                  "(w p j) d -> p w j d", p=128, j=8)
                              nc.sync.dma_start(dst, ys[:].rearrange(
                                  "p (w j d) -> p w j d", j=8, d=D))
                      # y_own: this core's nodes, [128, cols*D] f16; group g
                      # stacks tiles 8g..8g+7 (K rows c'*3+f), out col c'*32+d
                      otile = npool.tile([24, ngo * 128], F32R, name="ochunk",
                                         tag="nch")
                      nc.sync.dma_start(otile[:], ownK[:])
                      for g in range(ngo):
                          nt_g = min(8, cfg.cols - g * 8)
                          ps = ppool.tile([128, 512], F32, name=f"pso{g}",
                                          tag="pso", space="PSUM")
                          nc.tensor.matmul(
                              ps[:, :nt_g * D],
                              otile[:3 * nt_g, g * 128:(g + 1) * 128],
                              wt[:3 * nt_g, :nt_g * D],
                              start=True, stop=True)
                          nc.scalar.copy(yown[:, g * 8 * D:(g * 8 + nt_g) * D],
                                         ps[:, :nt_g * D])

                # ---------- Phase B: gather + select + max tree ----------
                for _rb in range(repeat_b):
                  with (
                      tc.tile_pool(name="bg", bufs=gbufs) as gpool,
                      tc.tile_pool(name="bv", bufs=cbufs) as vpool,
                      tc.tile_pool(name="btree", bufs=cbufs) as tpool,
                  ):
                      for t in range(cfg.nt):
                          base = t * (iw + mw)
                          it = imt_all[:][:, base:base + iw]
                          mt = imt_all[:].bitcast(F16)[:, base + iw:base + iw + mw]
                          g_t = gpool.tile([128, cfg.cblk * 4 * D], F16,
                                           name=f"g{t}", tag="g")
                          if skip_gather:
                              nc.sync.dma_start(
                                  g_t[:].bitcast(I16)[:, :cfg.nidx // 16],
                                  imsk_d[:, :iw])
                          else:
                              nc.gpsimd.dma_gather(
                                  out_ap=g_t[:].rearrange("p (c e) -> p c e", e=4 * D),
                                  in_ap=ytab[:],
                                  idxs_ap=it,
                                  num_idxs=cfg.nidx,
                                  num_idxs_reg=cfg.nidx,
                                  elem_size=4 * D,
                                  single_packet=False,
                                  queue_num=t % 4,
                              )
                          if not skip_dve:
                              # [p, (c g), d2, dp] views: innermost contiguous
                              # pair on every operand -> DVE 2x mode
                              gvp = g_t[:].rearrange(
                                  "p (cg d2 dp) -> p cg d2 dp", d2=D // 2, dp=2)
                              biasp = mt.rearrange(
                                  "p (cg o dp) -> p cg o dp", o=1, dp=2) \
                                  .broadcast_to([128, cfg.cblk * 4, D // 2, 2])
                              gb = vpool.tile([128, cfg.cblk * 4 * D], F16,
                                              name=f"gb{t}", tag="gb")
                              gbp = gb[:].rearrange(
                                  "p (cg d2 dp) -> p cg d2 dp", d2=D // 2, dp=2)
                              nc.vector.tensor_tensor(gbp, gvp, biasp,
                                                      op=mybir.AluOpType.add)
                              gbv = gb[:].rearrange("p (c g d) -> p c g d", g=4, d=D)
                              s2 = vpool.tile([128, cfg.cblk * 2 * D], F16,
                                              name=f"s2{t}", tag="s2")
                              s2v = s2[:].rearrange("p (c g d) -> p c g d", g=2, d=D)
                              nc.vector.tensor_tensor(s2v, gbv[:, :, 0:2, :],
                                                      gbv[:, :, 2:4, :],
                                                      op=mybir.AluOpType.max)
                              v = vpool.tile([128, cfg.cblk * D], F16, name=f"v{t}", tag="v")
                              vv = v[:].rearrange("p (c d) -> p c d", d=D)
                              nc.vector.tensor_tensor(vv, s2v[:, :, 0, :], s2v[:, :, 1, :],
                                                      op=mybir.AluOpType.max)
                              v4 = v[:].rearrange("p (m k d) -> p m k d", k=DEGREE, d=D)
                              t1_ = tpool.tile([128, cfg.mblk * 8 * D], F16,
                                               name=f"t1_{t}", tag="t1")
                              t1v = t1_[:].rearrange("p (m k d) -> p m k d", k=8, d=D)
                              nc.vector.tensor_tensor(t1v, v4[:, :, 0:8, :],
                                                      v4[:, :, 8:16, :],
                                                      op=mybir.AluOpType.max)
                              t2_ = tpool.tile([128, cfg.mblk * 4 * D], F16,
                                               name=f"t2_{t}", tag="t2")
                              t2v = t2_[:].rearrange("p (m k d) -> p m k d", k=4, d=D)
                              nc.vector.tensor_tensor(t2v, t1v[:, :, 0:4, :],
                                                      t1v[:, :, 4:8, :],
                                                      op=mybir.AluOpType.max)
                              t3_ = tpool.tile([128, cfg.mblk * 2 * D], F16,
                                               name=f"t3_{t}", tag="t3")
                              t3v = t3_[:].rearrange("p (m k d) -> p m k d", k=2, d=D)
                              nc.vector.tensor_tensor(t3v, t2v[:, :, 0:2, :],
                                                      t2v[:, :, 2:4, :],
                                                      op=mybir.AluOpType.max)
                              mslice = maxg[:].rearrange("p (c d) -> p c d", d=D) \
                                  [:, t * cfg.mblk:(t + 1) * cfg.mblk, :]
                              nc.vector.tensor_tensor(mslice, t3v[:, :, 0, :],
                                                      t3v[:, :, 1, :],
                                                      op=mybir.AluOpType.max)

                # ---------------- Phase C: combine ----------------
                with tc.tile_pool(name="cpool", bufs=1) as cp:
                    wp = cp.tile([128, D], F16)
                    nc.sync.dma_start(wp[:], wphi[:])
                    pv = cp.tile([128, cfg.cols], F32)
                    nc.sync.dma_start(pv[:], prev_d[:])
                    diff = cp.tile([128, cfg.cols * D], F16)
                    nc.vector.tensor_tensor(diff[:], maxg[:], yown[:],
                                            op=mybir.AluOpType.subtract)
                    prod = cp.tile([128, cfg.cols * D], F16)
                    pvv = prod[:].rearrange("p (c d) -> p c d", d=D)
                    wpb = wp[:].rearrange("p (o d) -> p o d", o=1) \
                        .broadcast_to([128, cfg.cols, D])
                    nc.vector.tensor_tensor(
                        pvv, diff[:].rearrange("p (c d) -> p c d", d=D),
                        wpb, op=mybir.AluOpType.mult)
                    red = cp.tile([128, cfg.cols], F32)
                    nc.vector.tensor_reduce(
                        red[:].rearrange("p (c o) -> p c o", o=1), pvv,
                        axis=mybir.AxisListType.X, op=mybir.AluOpType.add)
                    outt = cp.tile([128, cfg.cols], F32)
                    nc.vector.tensor_tensor(outt[:], red[:], pv[:],
                                            op=mybir.AluOpType.add)
                    nc.sync.dma_start(out_d[:], outt[:])

    nc.compile()
    return nc


def host_prep(cfg: Cfg, edge_dst, nodes, W_theta, W_phi, prev, n_cores=N_CORES):
    """Build per-core input maps. Index preprocessing + padding only."""
    N = cfg.n_nodes
    dst = np.asarray(edge_dst).astype(np.int64).reshape(N, DEGREE)
    nodes = np.asarray(nodes, dtype=np.float32)
    npad = np.zeros((cfg.ntab, 3), dtype=np.float32)
    npad[:N] = nodes
    # K-stacked layout: row j*3+f, col w*128+p  <-  node 1024w+8p+j, feat f
    nodesK = np.ascontiguousarray(
        npad.reshape(cfg.ntab // 1024, 128, 8, 3)
        .transpose(2, 3, 0, 1).reshape(24, cfg.ntab // 8))
    wth = np.asarray(W_theta, dtype=np.float32)
    wtheta = np.zeros((24, 8 * OUT_DIM), dtype=np.float32)
    for j in range(8):
        wtheta[3 * j:3 * j + 3, OUT_DIM * j:OUT_DIM * (j + 1)] = wth
    wphi = np.tile(np.asarray(W_phi, dtype=np.float32)[None, :] / OUT_DIM,
                   (128, 1)).astype(np.float16)
    ngo = -(-cfg.cols // 8)

    in_maps = []
    for c in range(n_cores):
        lo = c * cfg.npc
        dst_c = np.zeros((cfg.npc_pad, DEGREE), dtype=np.int64)
        dst_c[:cfg.npc] = dst[lo:lo + cfg.npc]
        # A[t, m, k, p] = dst_c[tn*t + 128m + p, k]
        A = dst_c.reshape(cfg.nt, cfg.mblk, 128, DEGREE).transpose(0, 1, 3, 2)
        flat = A.reshape(cfg.nt, cfg.nidx)        # j = (m*DEG + k)*128 + p
        quad = (flat >> 2).astype(np.int16)
        sub = (flat & 3).astype(np.int16)
        iw = quad.reshape(cfg.nt, cfg.nidx // 16, 16).transpose(0, 2, 1)
        idx_w = np.tile(iw, (1, 8, 1)).astype(np.int16)
        sub_pc = sub.reshape(cfg.nt, cfg.cblk, 128).transpose(0, 2, 1)
        # bias[t, p, c, g] = 0 where g == dst&3 else -30000; duplicated over
        # feature-pairs so the kernel-side AP has a contiguous innermost dim
        bias = np.where(sub_pc[:, :, :, None] == np.arange(4)[None, None, None, :],
                        np.float16(0), np.float16(-30000))
        msk = np.repeat(bias.reshape(cfg.nt, 128, cfg.cblk * 4), 2,
                        axis=2).astype(np.float16)
        imsk = np.concatenate([idx_w, msk.view(np.int16)], axis=2) \
            .transpose(1, 0, 2).reshape(128, -1)
        own = np.zeros((ngo * 1024, 3), dtype=np.float32)
        hi = min(lo + cfg.npc_pad, N)
        own[:hi - lo] = nodes[lo:hi]
        # group g stacks tiles 8g..8g+7: row c'*3+f, col g*128+p
        ownK = np.ascontiguousarray(
            own.reshape(ngo, 8, 128, 3).transpose(1, 3, 0, 2)
            .reshape(24, ngo * 128))
        prevp = np.zeros((128, cfg.cols), dtype=np.float32)
        pcv = np.zeros(cfg.npc_pad, dtype=np.float32)
        pcv[:cfg.npc] = prev[lo:lo + cfg.npc]
        prevp[:, :] = pcv.reshape(cfg.cols, 128).T
        in_maps.append({
            "nodesK": nodesK, "ownK": ownK,
            "wtheta": wtheta, "wphi": wphi,
            "imsk": np.ascontiguousarray(imsk),
            "prevp": prevp,
        })
    return in_maps


def assemble(cfg: Cfg, results, n_cores=N_CORES):
    out = np.empty(n_cores * cfg.npc, dtype=np.float32)
    for c in range(n_cores):
        arr = results[c]["outp"]            # [128, cols]
        out[c * cfg.npc:(c + 1) * cfg.npc] = arr.T.ravel()[:cfg.npc]
    return out


_CACHE = {}


def _make_exec(cfg: Cfg, n_cores=N_CORES, **build_kw):
    """Compile once and build a reusable jitted executable for the SPMD run."""
    import jax
    from jax.sharding import Mesh, PartitionSpec
    from jax.experimental.shard_map import shard_map
    from concourse import bass2jax

    nc = build_bass(cfg, n_cores, **build_kw)
    bass2jax.install_neuronx_cc_hook()
    in_names, out_names, out_avals, zero_shapes = [], [], [], []
    partition_name = nc.partition_id_tensor.name if nc.partition_id_tensor else None
    for alloc in nc.m.functions[0].allocations:
        if not isinstance(alloc, mybir.MemoryLocationSet):
            continue
        name = alloc.memorylocations[0].name
        if alloc.kind == "ExternalInput":
            if name != partition_name:
                in_names.append(name)
        elif alloc.kind == "ExternalOutput":
            out_names.append(name)
            shape = tuple(alloc.tensor_shape)
            dtype = mybir.dt.np(alloc.dtype)
            out_avals.append(jax.core.ShapedArray(shape, dtype))
            zero_shapes.append((shape, dtype))
    n_params = len(in_names)
    n_outs = len(out_avals)
    in_names_all = in_names + out_names
    if partition_name is not None:
        in_names_all.append(partition_name)
    donate = tuple(range(n_params, n_params + n_outs))

    def _body(*args):
        operands = list(args)
        if partition_name is not None:
            operands.append(bass2jax.partition_id_tensor())
        outs = bass2jax._bass_exec_p.bind(
            *operands,
            out_avals=tuple(out_avals),
            in_names=tuple(in_names_all),
            out_names=tuple(out_names),
            lowering_input_output_aliases=(),
            sim_require_finite=True,
            sim_require_nnan=True,
            nc=nc,
        )
        return tuple(outs)

    devices = jax.devices()[:n_cores]
    mesh = Mesh(np.asarray(devices), ("core",))
    in_specs = (PartitionSpec("core"),) * (n_params + n_outs)
    out_specs = (PartitionSpec("core"),) * len(out_names)
    sharded = jax.jit(
        shard_map(_body, mesh=mesh, in_specs=in_specs, out_specs=out_specs,
                  check_rep=False),
        donate_argnums=donate, keep_unused=True,
    )

    def run(in_maps):
        concat_in = [np.concatenate([np.asarray(m[name]) for m in in_maps], axis=0)
                     for name in in_names]
        zeros = [np.zeros((n_cores * s[0], *s[1:]), d) for s, d in zero_shapes]
        outs = sharded(*concat_in, *zeros)
        return [
            {name: np.asarray(outs[i]).reshape(n_cores, *out_avals[i].shape)[c]
             for i, name in enumerate(out_names)}
            for c in range(n_cores)
        ]

    return run


def _get_exec(cfg: Cfg, n_cores=N_CORES, **build_kw):
    key = (cfg.n_nodes, cfg.npc, cfg.tn, n_cores, tuple(sorted(build_kw.items())))
    if key not in _CACHE:
        _CACHE[key] = _make_exec(cfg, n_cores, **build_kw)
    return _CACHE[key]


def kernel(previous_inclusion_score, nodes, W_phi, W_theta, edge_src, edge_dst):
    cfg = Cfg()
    src = np.asarray(edge_src).astype(np.int64)
    expect = np.repeat(np.arange(N_NODES, dtype=np.int64), DEGREE)
    assert np.array_equal(src, expect), "edge_src structure mismatch"
    run = _get_exec(cfg)
    in_maps = host_prep(cfg, edge_dst, nodes, W_theta, W_phi,
                        np.asarray(previous_inclusion_score, dtype=np.float32))
    results = run(in_maps)
    return assemble(cfg, results)

